# revision 44
# baseline (speedup 1.0000x reference)
"""Multi-head causal attention on 8 TRN2 NeuronCores (v2).

Sharding: core c -> (b = c // 4, hg = c % 4). Data parallel over batch
(B=2), tensor parallel over heads (16 -> 4 groups of 4). Each core
computes q/k/v projections for its 4 heads on its batch row, full causal
attention, and a partial output projection through its 256-row slice of
Wp. The host sums the 4 head-group partials per batch and adds bp.

v2 changes vs the original baseline:
- Host passes x pre-transposed AND pre-cast to bf16 in the exact SBUF
  layout ([4 stages, 128 part, 8 cc, 512 t]); weights likewise bf16 in
  stationary-ready layouts. This removes all on-chip transposes
  (PE matmuls + evac copies), all fp32->bf16 casts, and halves the
  input DMA bytes. bv arrives pre-broadcast [128, 256].
- Scores are emitted per head-PAIR at the same key block: head 2p uses
  kTc/qTc partitions 0:64, head 2p+1 uses 64:128, so the two K=64
  matmuls land in PE row-groups 0 and 64 (tile_position auto-derived
  from base_partition) and run CONCURRENTLY in the array (~2x on the
  scores). One ScalarE exp covers both heads' 512-col strips.
- PSUM: mmps 1 bank + scores 2x2 banks + y-accum 3 banks = 8. The
  y/denominator PSUM tile is evacuated to SBUF in one copy right after
  the last PV so the bank frees before the (slow) normalize chain.
- Output written bf16 (host accumulates partials in fp32); out DMA per
  512-col half right after its evacuation. For the final q-chunk the
  output projection is split per head-pair: pair-0 partials drip during
  pair-1's attention, so the tail after the last PV is only 8 short
  matmul+add+DMA steps.
- ScalarE does ONLY exp (q bias moved to DVE tensor_scalar); the exp
  table is pre-warmed during the initial DMA window.
"""

import numpy as np
import ml_dtypes

import concourse.bass as bass
import concourse.mybir as mybir
import concourse.tile as tile
from concourse import bacc
from concourse.bass_utils import run_bass_kernel_spmd

F32 = mybir.dt.float32
BF16 = mybir.dt.bfloat16
BF16_NP = ml_dtypes.bfloat16

B, T, C, H = 2, 2048, 1024, 16
NCORES = 8
HG = 4            # head groups (tensor-parallel degree)
NH = H // HG      # heads per core = 4
HD = C // H       # head dim = 64
HS = NH * HD      # head-slice width per core = 256
SCALE = 1.0 / float(np.sqrt(HD))

TB = T // 128     # 16 row blocks
CCH = C // 128    # 8 contraction chunks
QC = T // 512     # 4 q chunks of 512
NP2 = NH // 2     # head pairs per core = 2


def _body(tc):
    nc = tc.nc
    x = nc.dram_tensor("x", [QC, 128, CCH, 512], BF16, kind="ExternalInput").ap()
    wq = nc.dram_tensor("wq", [128, CCH, HS], BF16, kind="ExternalInput").ap()
    wk = nc.dram_tensor("wk", [128, CCH, HS], BF16, kind="ExternalInput").ap()
    wv = nc.dram_tensor("wv", [128, CCH, HS], BF16, kind="ExternalInput").ap()
    wp = nc.dram_tensor("wp", [128, HS // 128, C], BF16, kind="ExternalInput").ap()
    bq = nc.dram_tensor("bq", [128, 2], F32, kind="ExternalInput").ap()
    bk = nc.dram_tensor("bk", [128, 2], F32, kind="ExternalInput").ap()
    bv = nc.dram_tensor("bv", [128, HS], F32, kind="ExternalInput").ap()
    out = nc.dram_tensor("out", [T, C], BF16, kind="ExternalOutput").ap()

    with (
        tc.tile_pool(name="const", bufs=1) as const,
        tc.tile_pool(name="persist", bufs=1) as persist,
        tc.tile_pool(name="expp", bufs=4) as expp,
        tc.tile_pool(name="workn", bufs=2) as workn,
        tc.tile_pool(name="osbp", bufs=4) as osbp,
        tc.tile_pool(name="obfp", bufs=3) as obfp,
        tc.tile_pool(name="mmps", bufs=1, space="PSUM") as mmps,
        tc.tile_pool(name="sps2", bufs=2, space="PSUM") as spsp,
        tc.tile_pool(name="yps", bufs=3, space="PSUM") as ypsp,
    ):
        # ---- input DMAs first so the sync queue starts pumping --------
        xT = [persist.tile([128, CCH, 512], BF16, tag=f"xT{t4}", name=f"xT{t4}")
              for t4 in range(QC)]
        wq_b = persist.tile([128, CCH, HS], BF16, tag="wq_b")
        wk_b = persist.tile([128, CCH, HS], BF16, tag="wk_b")
        wv_b = persist.tile([128, CCH, HS], BF16, tag="wv_b")
        wp_b = persist.tile([128, HS // 128, C], BF16, tag="wp_b")
        bq_sb = const.tile([128, 2], F32, tag="bq_sb")
        bk_sb = const.tile([128, 2], F32, tag="bk_sb")
        bv_bc = persist.tile([128, HS], F32, tag="bv_bc")

        nc.sync.dma_start(xT[0][:], x[0])
        nc.sync.dma_start(wq_b[:], wq)
        nc.sync.dma_start(bq_sb[:], bq)
        nc.sync.dma_start(bk_sb[:], bk)
        nc.sync.dma_start(wk_b[:], wk)
        nc.sync.dma_start(wv_b[:], wv)
        nc.sync.dma_start(bv_bc[:], bv)
        nc.sync.dma_start(xT[1][:], x[1])
        nc.sync.dma_start(wp_b[:], wp)
        nc.sync.dma_start(xT[2][:], x[2])
        nc.sync.dma_start(xT[3][:], x[3])

        # No HAM warmup matmuls: the ~7.8us framework preamble already
        # covers the initial DMA window, so warmups would only burn cold
        # cycles. Preload the Exp table while DMA is still streaming.
        warm_in = const.tile([128, 512], BF16, tag="warm_in")
        nc.gpsimd.memset(warm_in[:], 0.0)
        wtmp = const.tile([128, 512], BF16, tag="wtmp")
        nc.scalar.activation(wtmp[:], warm_in[:],
                             mybir.ActivationFunctionType.Exp, scale=1.0)

        # 0/1 lower-triangular mask (keep (k, q) iff q >= k) for the
        # diagonal 128-col strips, applied post-exp as a DVE multiply.
        # Two copies side by side so one op masks both heads of a pair.
        trimask2 = const.tile([128, 2, 128], BF16, tag="trimask2")
        nc.gpsimd.memset(trimask2[:], 1.0)
        for j in range(2):
            nc.gpsimd.affine_select(
                out=trimask2[:, j, :], in_=trimask2[:, j, :],
                compare_op=mybir.AluOpType.is_ge,
                fill=0.0, base=0, pattern=[[1, 128]], channel_multiplier=-1,
            )

        # v in natural layout [T, 4 heads x (64 + ones col)]
        v_sb = [persist.tile([128, 4, NH * 65], BF16, tag=f"v_sb{i}",
                             name=f"v_sb{i}") for i in range(4)]
        for i in range(4):
            nc.gpsimd.memset(
                v_sb[i][:].rearrange("p k (h e) -> p k h e", e=65)[:, :, :, 64:65], 1.0
            )

        qTc = [[persist.tile([128, 512], BF16, tag=f"qTc{p}_{t}", name=f"qTc{p}_{t}")
                for t in range(QC)] for p in range(NP2)]
        kTc = [[persist.tile([128, 512], BF16, tag=f"kTc{p}_{t}", name=f"kTc{p}_{t}")
                for t in range(QC)] for p in range(NP2)]
        yT = [persist.tile([128, 512], BF16, tag=f"yT{q}", name=f"yT{q}")
              for q in range(QC * 2)]  # index 2*qc + pair

        # ---- builders --------------------------------------------------
        def qk_one(t4, pair, which, ps=None):
            w_b, b_sb, dst = ((wq_b, bq_sb, qTc) if which == "q"
                              else (wk_b, bk_sb, kTc))
            if ps is None:
                ps = mmps.tile([128, 512], F32, tag="mm512",
                               name=f"{which}ps{pair}_{t4}")[:]
            for cc in range(CCH):
                nc.tensor.matmul(
                    ps,
                    w_b[:, cc, pair * 128 : (pair + 1) * 128],
                    xT[t4][:, cc, :],
                    start=(cc == 0),
                    stop=(cc == CCH - 1),
                )
            nc.vector.tensor_scalar_add(
                dst[pair][t4][:], ps, b_sb[:, pair : pair + 1]
            )

        def v_tb(tb, ps=None):
            if ps is None:
                ps = mmps.tile([128, 512], F32, tag="mm512", name=f"vps{tb}")[:]
            for cc in range(CCH):
                nc.tensor.matmul(
                    ps[:, :HS],
                    xT[tb // 4][:, cc, (tb % 4) * 128 : (tb % 4 + 1) * 128],
                    wv_b[:, cc, :],
                    start=(cc == 0),
                    stop=(cc == CCH - 1),
                )
            vdst = v_sb[tb // 4][:, tb % 4, :].rearrange(
                "p (h e) -> p h e", e=65)[:, :, 0:64]
            nc.vector.tensor_tensor(vdst, ps[:, :HS], bv_bc[:], mybir.AluOpType.add)

        slot_n = [0]

        def chain_slots():
            # back-to-back chain phases only (prologue / final S4, when the
            # scores pool is idle): rotate psum targets over the scores-pool
            # banks + mmps so chain n+1 never stalls on chain n's evacuation
            while True:
                slot_n[0] += 1
                t = spsp.tile([128, 2, 512], F32, tag="sps2",
                              name=f"slot{slot_n[0]}")
                yield t[:, 0, :]
                yield t[:, 1, :]
                slot_n[0] += 1
                yield mmps.tile([128, 512], F32, tag="mm512",
                                name=f"slot{slot_n[0]}")[:]

        # ---- attention units: (qc, pair, kb) ---------------------------
        units = []
        for qc in range(QC):
            for p in range(NP2):
                nkb = 4 * qc + 4
                for kb in range(nkb):
                    units.append((qc, p, kb, kb == nkb - 1))
        esbs = {}
        yps_tiles = {}

        def emit_scores(i):
            qc, p, kb, _ = units[i]
            d = max(0, 128 * (kb - 4 * qc))
            sps = spsp.tile([128, 2, 512], F32, tag="sps2", name=f"sps{i}")
            for j in range(2):
                off = 64 * j
                # head 2p on array row-group 0, head 2p+1 on row-group 64:
                # both matmuls run concurrently (tile_position from
                # base_partition)
                nc.tensor.matmul(
                    sps[:, j, d:512],
                    kTc[p][kb // 4][off : off + 64,
                                    (kb % 4) * 128 : (kb % 4 + 1) * 128],
                    qTc[p][qc][off : off + 64, d:512],
                    start=True, stop=True,
                )
            esb = expp.tile([128, 2, 512], BF16, tag="esb", name=f"esb{i}")
            # one activation per unit: a 3D AP [128, 2, 512-d] covers both
            # heads' valid strips even on the diagonal (no per-half split,
            # saving the 352-cycle fixed cost of a second ACT)
            nc.scalar.activation(
                esb[:, :, d:512], sps[:, :, d:512],
                mybir.ActivationFunctionType.Exp, scale=SCALE,
            )
            if kb >= 4 * qc:
                # zero the upper triangle of the diagonal 128-col strip
                # (both heads in one op)
                nc.vector.tensor_tensor(
                    esb[:, :, d : d + 128], esb[:, :, d : d + 128],
                    trimask2[:], mybir.AluOpType.mult,
                )
            esbs[i] = esb

        def emit_pv(i):
            qc, p, kb, is_last = units[i]
            d = max(0, 128 * (kb - 4 * qc))
            if kb == 0:
                for j in range(2):
                    yps_tiles[(qc, p, j)] = ypsp.tile(
                        [65, 512], F32, tag="yps", name=f"yps{qc}_{p}_{j}"
                    )
            esb = esbs.pop(i)
            for j in range(2):
                h = 2 * p + j
                nc.tensor.matmul(
                    yps_tiles[(qc, p, j)][:, d:512],
                    v_sb[kb // 4][:, kb % 4, 65 * h : 65 * h + 65],
                    esb[:, j, d:512],
                    start=(kb == 0),
                    stop=is_last,
                )
            if not is_last:
                return
            # evacuate PSUM (frees the banks), then normalize from SBUF.
            # NOTE: keep every DVE op partition-ALIGNED between in and out
            # (ops with a partition-base shift, like reciprocal reading a
            # row-64 slice, produce garbage on HW); the row-64 -> row-0
            # move must be a plain tensor_copy.
            yc = workn.tile([65, 2, 512], BF16, tag="ycop", name=f"yc{qc}_{p}")
            for j in range(2):
                nc.vector.tensor_copy(yc[:, j, :], yps_tiles[(qc, p, j)][:])
            for j in range(2):
                den = workn.tile([1, 512], F32, tag="den")
                nc.vector.tensor_copy(den[:], yps_tiles.pop((qc, p, j))[64:65, :])
                rec = workn.tile([1, 512], F32, tag="rec")
                nc.vector.reciprocal_approx_fast(rec[:], den[:])
                rbc = workn.tile([64, 512], F32, tag="rbc")
                nc.gpsimd.partition_broadcast(rbc[:], rec[:])
                nc.vector.tensor_tensor(
                    yT[2 * qc + p][64 * j : 64 * j + 64, :],
                    yc[0:64, j, :], rbc[:], mybir.AluOpType.mult,
                )

        # ---- output projection ----------------------------------------
        def emit_s4_qb(qc, qb):
            # combined (both pairs ready): 2 psum chains + evac + DMA halves
            osb = obfp.tile([128, C], BF16, tag="osb", name=f"osb{qb}")
            for cc2 in range(2):
                ps = mmps.tile([128, 512], F32, tag="mm512", name=f"ops{qb}_{cc2}")
                for ych in range(HS // 128):
                    nc.tensor.matmul(
                        ps[:],
                        yT[2 * qc + ych][:, (qb % 4) * 128 : (qb % 4 + 1) * 128],
                        wp_b[:, ych, cc2 * 512 : (cc2 + 1) * 512],
                        start=(ych == 0),
                        stop=(ych == HS // 128 - 1),
                    )
                dst = osb[:, cc2 * 512 : (cc2 + 1) * 512]
                if cc2 == 0:
                    nc.scalar.copy(dst, ps[:])
                else:
                    nc.vector.tensor_copy(dst, ps[:])
                nc.sync.dma_start(
                    out[qb * 128 : (qb + 1) * 128, cc2 * 512 : (cc2 + 1) * 512],
                    dst,
                )

        def s4_partial0(qc, qb, cc2):
            # pair-0 partial for the final q chunk (dripped during pair 1)
            ps = mmps.tile([128, 512], F32, tag="mm512", name=f"p0ps{qb}_{cc2}")
            nc.tensor.matmul(
                ps[:],
                yT[2 * qc][:, (qb % 4) * 128 : (qb % 4 + 1) * 128],
                wp_b[:, 0, cc2 * 512 : (cc2 + 1) * 512],
                start=True, stop=True,
            )
            o4 = osb4.setdefault(
                qb, osbp.tile([128, C], F32, tag="osb4", name=f"osb4_{qb}"))
            if cc2 == 0:
                nc.scalar.copy(o4[:, 0:512], ps[:])
            else:
                nc.vector.tensor_copy(o4[:, 512:1024], ps[:])

        def s4_final1(qc, qb, cc2, ps):
            nc.tensor.matmul(
                ps,
                yT[2 * qc + 1][:, (qb % 4) * 128 : (qb % 4 + 1) * 128],
                wp_b[:, 1, cc2 * 512 : (cc2 + 1) * 512],
                start=True, stop=True,
            )
            obf = obfp.tile([128, 512], BF16, tag="obf", name=f"obf{qb}_{cc2}")
            nc.vector.tensor_tensor(
                obf[:], ps, osb4[qb][:, cc2 * 512 : (cc2 + 1) * 512],
                mybir.AluOpType.add,
            )
            nc.sync.dma_start(
                out[qb * 128 : (qb + 1) * 128, cc2 * 512 : (cc2 + 1) * 512],
                obf[:],
            )

        # ---- pipelined walk -------------------------------------------
        # build steps are keyed by what they produce; scores/PV emission
        # "need()"s its inputs, which pops (in order) only as far as
        # required -- so the lookahead never force-flushes a whole stage
        built = set()
        build_q = []     # (key, closure)
        pending_s4 = []  # (qc, qb) combined-mode emissions
        final_steps = [] # closures for the final q chunk's split S4
        osb4 = {}

        def bq_pop():
            key, fn = build_q.pop(0)
            fn()
            built.add(key)

        def need(*keys):
            while not all(k in built for k in keys):
                bq_pop()

        def queue_stage(t4):
            # consumption order: p0 q/k, this stage's v blocks, p1 q/k
            build_q.append((("q", t4, 0), lambda t4=t4: qk_one(t4, 0, "q")))
            build_q.append((("k", t4, 0), lambda t4=t4: qk_one(t4, 0, "k")))
            for g in range(4):
                build_q.append((("v", 4 * t4 + g),
                                lambda t4=t4, g=g: v_tb(4 * t4 + g)))
            build_q.append((("q", t4, 1), lambda t4=t4: qk_one(t4, 1, "q")))
            build_q.append((("k", t4, 1), lambda t4=t4: qk_one(t4, 1, "k")))

        scores_done = 0

        def scores_ready(i):
            qc, p, kb, _ = units[i]
            return ("q", qc, p) in built and ("k", kb // 4, p) in built

        def advance_force(target):
            nonlocal scores_done
            while scores_done < min(target, len(units)):
                qc, p, kb, _ = units[scores_done]
                need(("q", qc, p), ("k", kb // 4, p))
                emit_scores(scores_done)
                scores_done += 1

        def advance_free(target):
            nonlocal scores_done
            while scores_done < min(target, len(units)) and scores_ready(scores_done):
                emit_scores(scores_done)
                scores_done += 1

        LOOKAHEAD = 3

        def walk():
            n_units = len(units)
            # process units in PAIRS: a burst of 2 score pairs then a burst
            # of 4 PV matmuls. Row-group width switches (64-row scores <->
            # 128-row PV) expose the PE drain, so halving the switch count
            # saves ~140ns per switch.
            for i, (qc, p, kb, is_last) in enumerate(units):
                if kb % 2 == 0:
                    advance_force(i + 2)
                    advance_free(i + 2 + LOOKAHEAD)
                    # ballast BETWEEN the scores burst and the PVs: the PVs
                    # wait on ScalarE's exp, and a stalled PV at the PE FIFO
                    # head blocks everything -- so fill the exp-latency
                    # window with work that has no exp dependency
                    if build_q:
                        bq_pop()
                    elif final_steps and i >= n_units - 8:
                        final_steps.pop(0)()
                    elif pending_s4:
                        emit_s4_qb(*pending_s4.pop(0))
                need(("v", kb))
                emit_pv(i)
                if kb % 2 == 1:
                    if build_q:
                        bq_pop()
                    elif final_steps and i >= n_units - 8:
                        final_steps.pop(0)()
                    elif pending_s4:
                        emit_s4_qb(*pending_s4.pop(0))
                if is_last:
                    if p == 0 and qc == QC - 1:
                        final_steps.extend(
                            (lambda qb=qb, cc2=cc2: s4_partial0(qc, qb, cc2))
                            for qb in range(4 * qc, 4 * qc + 4)
                            for cc2 in range(2)
                        )
                    elif p == 1 and qc < QC - 1:
                        pending_s4.extend(
                            (qc, qb) for qb in range(4 * qc, 4 * qc + 4))
                    elif p == 1 and qc == QC - 1:
                        while pending_s4:
                            emit_s4_qb(*pending_s4.pop(0))
                        while final_steps:
                            final_steps.pop(0)()
                        # scores pool is idle now: rotate the final partials
                        # over its banks so the matmul+add+DMA steps pipeline
                        slots = chain_slots()
                        for qb in range(4 * qc, 4 * qc + 4):
                            for cc2 in range(2):
                                s4_final1(qc, qb, cc2, next(slots))
                if kb == 0 and p == 0 and qc + 1 < QC:
                    queue_stage(qc + 1)

        # minimal inline prologue: just what the first units need (slot
        # rotation so the back-to-back chains never stall on evacuation);
        # the rest of stage 0 goes on the build queue
        slots = chain_slots()
        qk_one(0, 0, "q", ps=next(slots))
        qk_one(0, 0, "k", ps=next(slots))
        v_tb(0, ps=next(slots))
        built.update({("q", 0, 0), ("k", 0, 0), ("v", 0)})
        for g in (1, 2, 3):
            build_q.append((("v", g), lambda g=g: v_tb(g)))
        build_q.append((("q", 0, 1), lambda: qk_one(0, 1, "q")))
        build_q.append((("k", 0, 1), lambda: qk_one(0, 1, "k")))

        walk()


_NC = None


def _build():
    global _NC
    if _NC is None:
        nc = bacc.Bacc("TRN2", target_bir_lowering=False)
        with tile.TileContext(nc) as tc:
            _body(tc)
        nc.compile()
        _NC = nc
    return _NC


def _shard_inputs(x, Wq, bq, Wk, bk, Wv, bv, Wp, bp):
    f32 = lambda a: np.asarray(a, dtype=np.float32)
    x, Wq, Wk, Wv, Wp = f32(x), f32(Wq), f32(Wk), f32(Wv), f32(Wp)
    bq, bk, bv = f32(bq), f32(bk), f32(bv)

    def to_bf(a):
        return np.ascontiguousarray(a.astype(BF16_NP))

    # x[b] -> [4 stages, 128 part, 8 cc, 512 t] with [t4,p,cc,j] = x[b].T[cc*128+p, t4*512+j]
    xh = [to_bf(x[b].T.reshape(CCH, 128, QC, 512).transpose(2, 1, 0, 3))
          for b in range(B)]
    in_maps = []
    for c in range(NCORES):
        b, hg = divmod(c, HG)
        cols = slice(hg * HS, (hg + 1) * HS)
        wq_l = to_bf(Wq[:, cols].reshape(CCH, 128, HS).transpose(1, 0, 2))
        wk_l = to_bf(Wk[:, cols].reshape(CCH, 128, HS).transpose(1, 0, 2))
        wv_l = to_bf(Wv[:, cols].reshape(CCH, 128, HS).transpose(1, 0, 2))
        wp_l = to_bf(Wp[cols, :].reshape(HS // 128, 128, C).transpose(1, 0, 2))
        in_maps.append({
            "x": xh[b],
            "wq": wq_l, "wk": wk_l, "wv": wv_l, "wp": wp_l,
            "bq": np.ascontiguousarray(bq[cols].reshape(2, 128).T),
            "bk": np.ascontiguousarray(bk[cols].reshape(2, 128).T),
            "bv": np.ascontiguousarray(np.broadcast_to(bv[cols], (128, HS))),
        })
    return in_maps


def run_sharded(inputs, **run_kwargs):
    """Compile (cached), run on cores 0-7, gather. Returns (out, results)."""
    nc = _build()
    in_maps = _shard_inputs(**inputs)
    res = run_bass_kernel_spmd(nc, in_maps, core_ids=list(range(NCORES)), **run_kwargs)
    out = np.zeros((B, T, C), np.float32)
    for c in range(NCORES):
        b = c // HG
        out[b] += np.asarray(res.results[c]["out"], dtype=np.float32)
    out += np.asarray(inputs["bp"], dtype=np.float32)
    return out, res


def kernel(x, Wq, bq, Wk, bk, Wv, bv, Wp, bp):
    out, _ = run_sharded(dict(
        x=x, Wq=Wq, bq=bq, Wk=Wk, bk=bk, Wv=Wv, bv=bv, Wp=Wp, bp=bp,
    ))
    return out


# revision 47
# speedup vs baseline: 1.2090x; 1.2090x over previous
"""Multi-head causal attention on 8 TRN2 NeuronCores (v2).

Sharding: core c -> (b = c // 4, hg = c % 4). Data parallel over batch
(B=2), tensor parallel over heads (16 -> 4 groups of 4). Each core
computes q/k/v projections for its 4 heads on its batch row, full causal
attention, and a partial output projection through its 256-row slice of
Wp. The host sums the 4 head-group partials per batch and adds bp.

v2 changes vs the original baseline:
- Host passes x pre-transposed AND pre-cast to bf16 in the exact SBUF
  layout ([4 stages, 128 part, 8 cc, 512 t]); weights likewise bf16 in
  stationary-ready layouts. This removes all on-chip transposes
  (PE matmuls + evac copies), all fp32->bf16 casts, and halves the
  input DMA bytes. bv arrives pre-broadcast [128, 256].
- Scores are emitted per head-PAIR at the same key block: head 2p uses
  kTc/qTc partitions 0:64, head 2p+1 uses 64:128, so the two K=64
  matmuls land in PE row-groups 0 and 64 (tile_position auto-derived
  from base_partition) and run CONCURRENTLY in the array (~2x on the
  scores). One ScalarE exp covers both heads' 512-col strips.
- PSUM: mmps 1 bank + scores 2x2 banks + y-accum 3 banks = 8. The
  y/denominator PSUM tile is evacuated to SBUF in one copy right after
  the last PV so the bank frees before the (slow) normalize chain.
- Output written bf16 (host accumulates partials in fp32); out DMA per
  512-col half right after its evacuation. For the final q-chunk the
  output projection is split per head-pair: pair-0 partials drip during
  pair-1's attention, so the tail after the last PV is only 8 short
  matmul+add+DMA steps.
- ScalarE does ONLY exp (q bias moved to DVE tensor_scalar); the exp
  table is pre-warmed during the initial DMA window.
"""

import numpy as np
import ml_dtypes

import concourse.bass as bass
import concourse.mybir as mybir
import concourse.tile as tile
from concourse import bacc
from concourse.bass_utils import run_bass_kernel_spmd

F32 = mybir.dt.float32
BF16 = mybir.dt.bfloat16
BF16_NP = ml_dtypes.bfloat16

B, T, C, H = 2, 2048, 1024, 16
NCORES = 8
HG = 4            # head groups (tensor-parallel degree)
NH = H // HG      # heads per core = 4
HD = C // H       # head dim = 64
HS = NH * HD      # head-slice width per core = 256
SCALE = 1.0 / float(np.sqrt(HD))

TB = T // 128     # 16 row blocks
CCH = C // 128    # 8 contraction chunks
QC = T // 512     # 4 q chunks of 512
NP2 = NH // 2     # head pairs per core = 2


def _body(tc, biased):
    nc = tc.nc
    x = nc.dram_tensor("x", [QC, 128, CCH, 512], BF16, kind="ExternalInput").ap()
    wq = nc.dram_tensor("wq", [128, CCH, HS], BF16, kind="ExternalInput").ap()
    wk = nc.dram_tensor("wk", [128, CCH, HS], BF16, kind="ExternalInput").ap()
    wv = nc.dram_tensor("wv", [128, CCH, HS], BF16, kind="ExternalInput").ap()
    wp = nc.dram_tensor("wp", [128, HS // 128, C], BF16, kind="ExternalInput").ap()
    bq = nc.dram_tensor("bq", [128, 2], F32, kind="ExternalInput").ap()
    bk = nc.dram_tensor("bk", [128, 2], F32, kind="ExternalInput").ap()
    bv = nc.dram_tensor("bv", [128, HS], F32, kind="ExternalInput").ap()
    out = nc.dram_tensor("out", [T, C], BF16, kind="ExternalOutput").ap()

    with (
        tc.tile_pool(name="const", bufs=1) as const,
        tc.tile_pool(name="persist", bufs=1) as persist,
        tc.tile_pool(name="expp", bufs=4) as expp,
        tc.tile_pool(name="workn", bufs=2) as workn,
        tc.tile_pool(name="osbp", bufs=4) as osbp,
        tc.tile_pool(name="obfp", bufs=3) as obfp,
        tc.tile_pool(name="mmps", bufs=1, space="PSUM") as mmps,
        tc.tile_pool(name="sps2", bufs=2, space="PSUM") as spsp,
        tc.tile_pool(name="yps", bufs=3, space="PSUM") as ypsp,
    ):
        # ---- input DMAs first so the sync queue starts pumping --------
        xT = [persist.tile([128, CCH, 512], BF16, tag=f"xT{t4}", name=f"xT{t4}")
              for t4 in range(QC)]
        wq_b = persist.tile([128, CCH, HS], BF16, tag="wq_b")
        wk_b = persist.tile([128, CCH, HS], BF16, tag="wk_b")
        wv_b = persist.tile([128, CCH, HS], BF16, tag="wv_b")
        wp_b = persist.tile([128, HS // 128, C], BF16, tag="wp_b")
        nc.sync.dma_start(xT[0][:], x[0])
        nc.sync.dma_start(wq_b[:], wq)
        if biased:
            bq_sb = const.tile([128, 2], F32, tag="bq_sb")
            bk_sb = const.tile([128, 2], F32, tag="bk_sb")
            bv_bc = persist.tile([128, HS], F32, tag="bv_bc")
            nc.sync.dma_start(bq_sb[:], bq)
            nc.sync.dma_start(bk_sb[:], bk)
        nc.sync.dma_start(wk_b[:], wk)
        nc.sync.dma_start(wv_b[:], wv)
        if biased:
            nc.sync.dma_start(bv_bc[:], bv)
        nc.sync.dma_start(xT[1][:], x[1])
        nc.sync.dma_start(wp_b[:], wp)
        nc.sync.dma_start(xT[2][:], x[2])
        nc.sync.dma_start(xT[3][:], x[3])

        # No HAM warmup matmuls: the ~7.8us framework preamble already
        # covers the initial DMA window, so warmups would only burn cold
        # cycles. Preload the Exp table while DMA is still streaming.
        warm_in = const.tile([128, 512], BF16, tag="warm_in")
        nc.gpsimd.memset(warm_in[:], 0.0)
        wtmp = const.tile([128, 512], BF16, tag="wtmp")
        nc.scalar.activation(wtmp[:], warm_in[:],
                             mybir.ActivationFunctionType.Exp, scale=1.0)

        # 0/1 lower-triangular mask (keep (k, q) iff q >= k) for the
        # diagonal 128-col strips, applied post-exp as a DVE multiply.
        # Two copies side by side so one op masks both heads of a pair.
        trimask2 = const.tile([128, 2, 128], BF16, tag="trimask2")
        nc.gpsimd.memset(trimask2[:], 1.0)
        for j in range(2):
            nc.gpsimd.affine_select(
                out=trimask2[:, j, :], in_=trimask2[:, j, :],
                compare_op=mybir.AluOpType.is_ge,
                fill=0.0, base=0, pattern=[[1, 128]], channel_multiplier=-1,
            )

        # v in natural layout [T, 4 heads x (64 + ones col)]
        v_sb = [persist.tile([128, 4, NH * 65], BF16, tag=f"v_sb{i}",
                             name=f"v_sb{i}") for i in range(4)]
        for i in range(4):
            nc.gpsimd.memset(
                v_sb[i][:].rearrange("p k (h e) -> p k h e", e=65)[:, :, :, 64:65], 1.0
            )

        qTc = [[persist.tile([128, 512], BF16, tag=f"qTc{p}_{t}", name=f"qTc{p}_{t}")
                for t in range(QC)] for p in range(NP2)]
        kTc = [[persist.tile([128, 512], BF16, tag=f"kTc{p}_{t}", name=f"kTc{p}_{t}")
                for t in range(QC)] for p in range(NP2)]
        yT = [persist.tile([128, 512], BF16, tag=f"yT{q}", name=f"yT{q}")
              for q in range(QC * 2)]  # index 2*qc + pair

        # ---- builders --------------------------------------------------
        def qk_one(t4, pair, which, ps=None):
            w_b, dst = (wq_b, qTc) if which == "q" else (wk_b, kTc)
            b_sb = (bq_sb if which == "q" else bk_sb) if biased else None
            if ps is None:
                ps = mmps.tile([128, 512], F32, tag="mm512",
                               name=f"{which}ps{pair}_{t4}")[:]
            for cc in range(CCH):
                nc.tensor.matmul(
                    ps,
                    w_b[:, cc, pair * 128 : (pair + 1) * 128],
                    xT[t4][:, cc, :],
                    start=(cc == 0),
                    stop=(cc == CCH - 1),
                )
            if biased:
                nc.vector.tensor_scalar_add(
                    dst[pair][t4][:], ps, b_sb[:, pair : pair + 1]
                )
            else:
                nc.vector.tensor_copy(dst[pair][t4][:], ps)

        def v_tb(tb, ps=None):
            if ps is None:
                ps = mmps.tile([128, 512], F32, tag="mm512", name=f"vps{tb}")[:]
            for cc in range(CCH):
                nc.tensor.matmul(
                    ps[:, :HS],
                    xT[tb // 4][:, cc, (tb % 4) * 128 : (tb % 4 + 1) * 128],
                    wv_b[:, cc, :],
                    start=(cc == 0),
                    stop=(cc == CCH - 1),
                )
            vdst = v_sb[tb // 4][:, tb % 4, :].rearrange(
                "p (h e) -> p h e", e=65)[:, :, 0:64]
            if biased:
                nc.vector.tensor_tensor(vdst, ps[:, :HS], bv_bc[:],
                                        mybir.AluOpType.add)
            else:
                nc.vector.tensor_copy(vdst, ps[:, :HS])

        slot_n = [0]

        def chain_slots():
            # back-to-back chain phases only (prologue / final S4, when the
            # scores pool is idle): rotate psum targets over the scores-pool
            # banks + mmps so chain n+1 never stalls on chain n's evacuation
            while True:
                slot_n[0] += 1
                t = spsp.tile([128, 2, 512], F32, tag="sps2",
                              name=f"slot{slot_n[0]}")
                yield t[:, 0, :]
                yield t[:, 1, :]
                slot_n[0] += 1
                yield mmps.tile([128, 512], F32, tag="mm512",
                                name=f"slot{slot_n[0]}")[:]

        # ---- attention units: (qc, pair, kb) ---------------------------
        units = []
        for qc in range(QC):
            for p in range(NP2):
                nkb = 4 * qc + 4
                for kb in range(nkb):
                    units.append((qc, p, kb, kb == nkb - 1))
        esbs = {}
        yps_tiles = {}

        def emit_scores(i):
            qc, p, kb, _ = units[i]
            d = max(0, 128 * (kb - 4 * qc))
            sps = spsp.tile([128, 2, 512], F32, tag="sps2", name=f"sps{i}")
            for j in range(2):
                off = 64 * j
                # head 2p on array row-group 0, head 2p+1 on row-group 64:
                # both matmuls run concurrently (tile_position from
                # base_partition)
                nc.tensor.matmul(
                    sps[:, j, d:512],
                    kTc[p][kb // 4][off : off + 64,
                                    (kb % 4) * 128 : (kb % 4 + 1) * 128],
                    qTc[p][qc][off : off + 64, d:512],
                    start=True, stop=True,
                )
            esb = expp.tile([128, 2, 512], BF16, tag="esb", name=f"esb{i}")
            # one activation per unit: a 3D AP [128, 2, 512-d] covers both
            # heads' valid strips even on the diagonal (no per-half split,
            # saving the 352-cycle fixed cost of a second ACT)
            nc.scalar.activation(
                esb[:, :, d:512], sps[:, :, d:512],
                mybir.ActivationFunctionType.Exp, scale=SCALE,
            )
            if kb >= 4 * qc:
                # zero the upper triangle of the diagonal 128-col strip
                # (both heads in one op)
                nc.vector.tensor_tensor(
                    esb[:, :, d : d + 128], esb[:, :, d : d + 128],
                    trimask2[:], mybir.AluOpType.mult,
                )
            esbs[i] = esb

        def emit_pv(i):
            qc, p, kb, is_last = units[i]
            d = max(0, 128 * (kb - 4 * qc))
            if kb == 0:
                for j in range(2):
                    yps_tiles[(qc, p, j)] = ypsp.tile(
                        [65, 512], F32, tag="yps", name=f"yps{qc}_{p}_{j}"
                    )
            esb = esbs.pop(i)
            for j in range(2):
                h = 2 * p + j
                nc.tensor.matmul(
                    yps_tiles[(qc, p, j)][:, d:512],
                    v_sb[kb // 4][:, kb % 4, 65 * h : 65 * h + 65],
                    esb[:, j, d:512],
                    start=(kb == 0),
                    stop=is_last,
                )
            if not is_last:
                return
            # evacuate PSUM (frees the banks), then normalize from SBUF.
            # NOTE: keep every DVE op partition-ALIGNED between in and out
            # (ops with a partition-base shift, like reciprocal reading a
            # row-64 slice, produce garbage on HW); the row-64 -> row-0
            # move must be a plain tensor_copy.
            yc = workn.tile([65, 2, 512], BF16, tag="ycop", name=f"yc{qc}_{p}")
            for j in range(2):
                nc.vector.tensor_copy(yc[:, j, :], yps_tiles[(qc, p, j)][:])
            for j in range(2):
                den = workn.tile([1, 512], F32, tag="den")
                nc.vector.tensor_copy(den[:], yps_tiles.pop((qc, p, j))[64:65, :])
                rec = workn.tile([1, 512], F32, tag="rec")
                nc.vector.reciprocal_approx_fast(rec[:], den[:])
                rbc = workn.tile([64, 512], F32, tag="rbc")
                nc.gpsimd.partition_broadcast(rbc[:], rec[:])
                nc.vector.tensor_tensor(
                    yT[2 * qc + p][64 * j : 64 * j + 64, :],
                    yc[0:64, j, :], rbc[:], mybir.AluOpType.mult,
                )

        # ---- output projection ----------------------------------------
        def emit_s4_qb(qc, qb):
            # combined (both pairs ready): 2 psum chains + evac + DMA halves
            osb = obfp.tile([128, C], BF16, tag="osb", name=f"osb{qb}")
            for cc2 in range(2):
                ps = mmps.tile([128, 512], F32, tag="mm512", name=f"ops{qb}_{cc2}")
                for ych in range(HS // 128):
                    nc.tensor.matmul(
                        ps[:],
                        yT[2 * qc + ych][:, (qb % 4) * 128 : (qb % 4 + 1) * 128],
                        wp_b[:, ych, cc2 * 512 : (cc2 + 1) * 512],
                        start=(ych == 0),
                        stop=(ych == HS // 128 - 1),
                    )
                dst = osb[:, cc2 * 512 : (cc2 + 1) * 512]
                if cc2 == 0:
                    nc.scalar.copy(dst, ps[:])
                else:
                    nc.vector.tensor_copy(dst, ps[:])
                nc.sync.dma_start(
                    out[qb * 128 : (qb + 1) * 128, cc2 * 512 : (cc2 + 1) * 512],
                    dst,
                )

        def s4_partial0(qc, qb, cc2):
            # pair-0 partial for the final q chunk (dripped during pair 1)
            ps = mmps.tile([128, 512], F32, tag="mm512", name=f"p0ps{qb}_{cc2}")
            nc.tensor.matmul(
                ps[:],
                yT[2 * qc][:, (qb % 4) * 128 : (qb % 4 + 1) * 128],
                wp_b[:, 0, cc2 * 512 : (cc2 + 1) * 512],
                start=True, stop=True,
            )
            o4 = osb4.setdefault(
                qb, osbp.tile([128, C], F32, tag="osb4", name=f"osb4_{qb}"))
            if cc2 == 0:
                nc.scalar.copy(o4[:, 0:512], ps[:])
            else:
                nc.vector.tensor_copy(o4[:, 512:1024], ps[:])

        def s4_final1(qc, qb, cc2, ps):
            nc.tensor.matmul(
                ps,
                yT[2 * qc + 1][:, (qb % 4) * 128 : (qb % 4 + 1) * 128],
                wp_b[:, 1, cc2 * 512 : (cc2 + 1) * 512],
                start=True, stop=True,
            )
            obf = obfp.tile([128, 512], BF16, tag="obf", name=f"obf{qb}_{cc2}")
            nc.vector.tensor_tensor(
                obf[:], ps, osb4[qb][:, cc2 * 512 : (cc2 + 1) * 512],
                mybir.AluOpType.add,
            )
            nc.sync.dma_start(
                out[qb * 128 : (qb + 1) * 128, cc2 * 512 : (cc2 + 1) * 512],
                obf[:],
            )

        # ---- pipelined walk -------------------------------------------
        # build steps are keyed by what they produce; scores/PV emission
        # "need()"s its inputs, which pops (in order) only as far as
        # required -- so the lookahead never force-flushes a whole stage
        built = set()
        build_q = []     # (key, closure)
        pending_s4 = []  # (qc, qb) combined-mode emissions
        final_steps = [] # closures for the final q chunk's split S4
        osb4 = {}

        def bq_pop():
            key, fn = build_q.pop(0)
            fn()
            built.add(key)

        def need(*keys):
            while not all(k in built for k in keys):
                bq_pop()

        def queue_stage(t4):
            # consumption order: p0 q/k, this stage's v blocks, p1 q/k
            build_q.append((("q", t4, 0), lambda t4=t4: qk_one(t4, 0, "q")))
            build_q.append((("k", t4, 0), lambda t4=t4: qk_one(t4, 0, "k")))
            for g in range(4):
                build_q.append((("v", 4 * t4 + g),
                                lambda t4=t4, g=g: v_tb(4 * t4 + g)))
            build_q.append((("q", t4, 1), lambda t4=t4: qk_one(t4, 1, "q")))
            build_q.append((("k", t4, 1), lambda t4=t4: qk_one(t4, 1, "k")))

        scores_done = 0

        def scores_ready(i):
            qc, p, kb, _ = units[i]
            return ("q", qc, p) in built and ("k", kb // 4, p) in built

        def advance_force(target):
            nonlocal scores_done
            while scores_done < min(target, len(units)):
                qc, p, kb, _ = units[scores_done]
                need(("q", qc, p), ("k", kb // 4, p))
                emit_scores(scores_done)
                scores_done += 1

        def advance_free(target):
            nonlocal scores_done
            while scores_done < min(target, len(units)) and scores_ready(scores_done):
                emit_scores(scores_done)
                scores_done += 1

        LOOKAHEAD = 3

        def walk():
            n_units = len(units)
            # process units in PAIRS: a burst of 2 score pairs then a burst
            # of 4 PV matmuls. Row-group width switches (64-row scores <->
            # 128-row PV) expose the PE drain, so halving the switch count
            # saves ~140ns per switch.
            for i, (qc, p, kb, is_last) in enumerate(units):
                if kb % 2 == 0:
                    advance_force(i + 2)
                    advance_free(i + 2 + LOOKAHEAD)
                    # ballast BETWEEN the scores burst and the PVs: the PVs
                    # wait on ScalarE's exp, and a stalled PV at the PE FIFO
                    # head blocks everything -- so fill the exp-latency
                    # window with work that has no exp dependency
                    if build_q:
                        bq_pop()
                    elif final_steps and i >= n_units - 8:
                        final_steps.pop(0)()
                    elif pending_s4:
                        emit_s4_qb(*pending_s4.pop(0))
                need(("v", kb))
                emit_pv(i)
                if kb % 2 == 1:
                    if build_q:
                        bq_pop()
                    elif final_steps and i >= n_units - 8:
                        final_steps.pop(0)()
                    elif pending_s4:
                        emit_s4_qb(*pending_s4.pop(0))
                if is_last:
                    if p == 0 and qc == QC - 1:
                        final_steps.extend(
                            (lambda qb=qb, cc2=cc2: s4_partial0(qc, qb, cc2))
                            for qb in range(4 * qc, 4 * qc + 4)
                            for cc2 in range(2)
                        )
                    elif p == 1 and qc < QC - 1:
                        pending_s4.extend(
                            (qc, qb) for qb in range(4 * qc, 4 * qc + 4))
                    elif p == 1 and qc == QC - 1:
                        while pending_s4:
                            emit_s4_qb(*pending_s4.pop(0))
                        while final_steps:
                            final_steps.pop(0)()
                        # scores pool is idle now: rotate the final partials
                        # over its banks so the matmul+add+DMA steps pipeline
                        slots = chain_slots()
                        for qb in range(4 * qc, 4 * qc + 4):
                            for cc2 in range(2):
                                s4_final1(qc, qb, cc2, next(slots))
                if kb == 0 and p == 0 and qc + 1 < QC:
                    queue_stage(qc + 1)

        # minimal inline prologue: just what the first units need (slot
        # rotation so the back-to-back chains never stall on evacuation);
        # the rest of stage 0 goes on the build queue
        slots = chain_slots()
        qk_one(0, 0, "q", ps=next(slots))
        qk_one(0, 0, "k", ps=next(slots))
        v_tb(0, ps=next(slots))
        built.update({("q", 0, 0), ("k", 0, 0), ("v", 0)})
        for g in (1, 2, 3):
            build_q.append((("v", g), lambda g=g: v_tb(g)))
        build_q.append((("q", 0, 1), lambda: qk_one(0, 1, "q")))
        build_q.append((("k", 0, 1), lambda: qk_one(0, 1, "k")))

        walk()


_NC = {}


def _build(biased=False):
    if biased not in _NC:
        nc = bacc.Bacc("TRN2", target_bir_lowering=False)
        with tile.TileContext(nc) as tc:
            _body(tc, biased)
        nc.compile()
        _NC[biased] = nc
    return _NC[biased]


def _shard_inputs(x, Wq, bq, Wk, bk, Wv, bv, Wp, bp):
    f32 = lambda a: np.asarray(a, dtype=np.float32)
    x, Wq, Wk, Wv, Wp = f32(x), f32(Wq), f32(Wk), f32(Wv), f32(Wp)
    bq, bk, bv = f32(bq), f32(bk), f32(bv)

    def to_bf(a):
        return np.ascontiguousarray(a.astype(BF16_NP))

    # x[b] -> [4 stages, 128 part, 8 cc, 512 t] with [t4,p,cc,j] = x[b].T[cc*128+p, t4*512+j]
    xh = [to_bf(x[b].T.reshape(CCH, 128, QC, 512).transpose(2, 1, 0, 3))
          for b in range(B)]
    in_maps = []
    for c in range(NCORES):
        b, hg = divmod(c, HG)
        cols = slice(hg * HS, (hg + 1) * HS)
        wq_l = to_bf(Wq[:, cols].reshape(CCH, 128, HS).transpose(1, 0, 2))
        wk_l = to_bf(Wk[:, cols].reshape(CCH, 128, HS).transpose(1, 0, 2))
        wv_l = to_bf(Wv[:, cols].reshape(CCH, 128, HS).transpose(1, 0, 2))
        wp_l = to_bf(Wp[cols, :].reshape(HS // 128, 128, C).transpose(1, 0, 2))
        in_maps.append({
            "x": xh[b],
            "wq": wq_l, "wk": wk_l, "wv": wv_l, "wp": wp_l,
            "bq": np.ascontiguousarray(bq[cols].reshape(2, 128).T),
            "bk": np.ascontiguousarray(bk[cols].reshape(2, 128).T),
            "bv": np.ascontiguousarray(np.broadcast_to(bv[cols], (128, HS))),
        })
    return in_maps


def run_sharded(inputs, **run_kwargs):
    """Compile (cached), run on cores 0-7, gather. Returns (out, results)."""
    biased = any(
        np.any(np.asarray(inputs[k])) for k in ("bq", "bk", "bv"))
    nc = _build(biased)
    in_maps = _shard_inputs(**inputs)
    if not biased:
        # unused ExternalInputs may be pruned from the NEFF; keep only
        # the tensors the chosen variant actually binds
        bound = {
            a.memorylocations[0].name
            for a in nc.m.functions[0].allocations
            if getattr(a, "memorylocations", None)
        }
        in_maps = [{k: v for k, v in m.items() if k in bound} for m in in_maps]
    res = run_bass_kernel_spmd(nc, in_maps, core_ids=list(range(NCORES)), **run_kwargs)
    out = np.zeros((B, T, C), np.float32)
    for c in range(NCORES):
        b = c // HG
        out[b] += np.asarray(res.results[c]["out"], dtype=np.float32)
    out += np.asarray(inputs["bp"], dtype=np.float32)
    return out, res


def kernel(x, Wq, bq, Wk, bk, Wv, bv, Wp, bp):
    out, _ = run_sharded(dict(
        x=x, Wq=Wq, bq=bq, Wk=Wk, bk=bk, Wv=Wv, bv=bv, Wp=Wp, bp=bp,
    ))
    return out
